# revision 4
# baseline (speedup 1.0000x reference)
"""DeepSeekMoE Trainium2 kernel (8 NeuronCores, data-parallel over tokens).

Problem: B=4, S=8192, H=576, I=512, E=8 routed experts (top-2) + 1 shared.
  y = shared_mlp(x) + sum_e w_e * expert_e_mlp(x),  w = renormalized top-2
  softmax router weights (dense-equivalent: non-selected experts get w=0).

Strategy:
  - Data-parallel: 32768 tokens split 4096/core across 8 cores.
  - Host pre-transposes x to [H, T] layout (H on partitions), padded
    H 576->640 (=5*128); row 576 is set to 1.0 so the router bias folds
    into the router matmul as an extra contraction row.
  - Router runs in exact fp32 (top-2 selection margins are as small as
    7.7e-6 in logit space); expert matmuls run in float32r (full PE rate
    at free-dim 512, ~1.6e-4 scale-relative error).
  - Top-2 on logits: m1/m2 via free-dim reduce_max + masking; renormalized
    weights via sigmoid(m1-m2) (softmax denominator cancels in the
    renormalization). Per-token weight rows are transposed via the PE and
    broadcast across partitions with a one-hot-selector matmul.
  - Per (half, expert): SwiGLU with PSUM accumulation over K-tiles; the
    routing weight is folded into h = silu(g)*u*w before the down matmul;
    y accumulates in SBUF across experts (shared expert initializes y).
"""
import numpy as np

NCORES = 8
B, S, H = 4, 8192, 576
I = 512
E = 8
T = B * S                 # 32768
TL = T // NCORES          # 4096 tokens per core
HP = 640                  # padded H (5*128); row 576 = bias row for router
KH = HP // 128            # 5 contraction tiles over H
KI = I // 128             # 4 contraction tiles over I
HT = HP // 128            # 5 output tiles over H
IT = I // 128             # 4 output tiles over I
CH = 512                  # token chunk (one PSUM bank at fp32)
NCH = TL // CH            # 8 chunks per core
NHALF = 2                 # token halves per core (weight streaming granularity)
HALF = TL // NHALF        # 2048
RC = TL // 128            # 32 router chunks

_SILU_SUB_SIGMOID = False  # CoreSim has no Silu LUT; tests substitute Sigmoid

_cached = {}


def _build_program():
    import concourse.tile as tile
    from concourse import bacc, mybir
    from concourse.masks import make_identity
    from contextlib import ExitStack

    f32 = mybir.dt.float32
    f32r = mybir.dt.float32r

    nc = bacc.Bacc("TRN2", target_bir_lowering=False, debug=False,
                   num_devices=NCORES)

    xtf_d = nc.dram_tensor("xtf", [128, KH, TL], f32, kind="ExternalInput").ap()
    xtr_d = nc.dram_tensor("xtr", [128, KH, TL], f32r, kind="ExternalInput").ap()
    wg_d = nc.dram_tensor("wgall", [E + 1, 128, KH, I], f32r, kind="ExternalInput").ap()
    wu_d = nc.dram_tensor("wuall", [E + 1, 128, KH, I], f32r, kind="ExternalInput").ap()
    wd_d = nc.dram_tensor("wdall", [E + 1, 128, KI, HP], f32r, kind="ExternalInput").ap()
    wr_d = nc.dram_tensor("wrf", [128, KH, E], f32, kind="ExternalInput").ap()
    sel_d = nc.dram_tensor("selmat", [E, E, 128], f32r, kind="ExternalInput").ap()
    yt_d = nc.dram_tensor("yt", [128, HT, TL], f32, kind="ExternalOutput").ap()

    with tile.TileContext(nc) as tc, ExitStack() as ctx:
        const = ctx.enter_context(tc.tile_pool(name="const", bufs=1))
        rpool = ctx.enter_context(tc.tile_pool(name="router", bufs=2))
        xpool = ctx.enter_context(tc.tile_pool(name="x", bufs=1))
        ypool = ctx.enter_context(tc.tile_pool(name="y", bufs=1))
        wpool = ctx.enter_context(tc.tile_pool(name="w", bufs=2))
        hpool = ctx.enter_context(tc.tile_pool(name="h", bufs=2))
        spool = ctx.enter_context(tc.tile_pool(name="s", bufs=2))

        # ---- constants
        wr_s = const.tile([128, KH, E], f32)
        nc.sync.dma_start(wr_s[:], wr_d[:])
        sel_s = const.tile([E, E, 128], f32r)
        nc.sync.dma_start(sel_s[:], sel_d[:])
        ident = const.tile([128, 128], f32)
        make_identity(nc, ident[:])

        # ---- phase R: router (exact fp32) + top-2 widget + transposes
        psum_r = tc.tile_pool(name="ps_r", bufs=1, space="PSUM")
        psum = psum_r.__enter__()
        logits = const.tile([128, RC, E], f32)
        for c in range(RC):
            xf = rpool.tile([128, KH, 128], f32, tag="xf")
            nc.sync.dma_start(xf[:], xtf_d[:, :, c * 128:(c + 1) * 128])
            lg = psum.tile([128, E], f32, name="lg")
            for k in range(KH):
                nc.tensor.matmul(lg[:], xf[:, k], wr_s[:, k],
                                 start=(k == 0), stop=(k == KH - 1))
            nc.vector.tensor_copy(logits[:, c], lg[:])

        shp = [128, RC, E]
        m1 = const.tile([128, RC, 1], f32)
        nc.vector.tensor_reduce(m1[:], logits[:], mybir.AxisListType.X,
                                mybir.AluOpType.max)
        nlt = const.tile(shp, f32)   # 1.0 where logit < m1
        nc.vector.tensor_tensor(nlt[:], logits[:], m1[:].to_broadcast(shp),
                                mybir.AluOpType.is_lt)
        t1 = const.tile(shp, f32)    # logits where below-max, else -1e30
        nc.vector.tensor_tensor(t1[:], logits[:], nlt[:], mybir.AluOpType.mult)
        t2 = const.tile(shp, f32)
        nc.vector.tensor_scalar(t2[:], nlt[:], 1e30, -1e30,
                                mybir.AluOpType.mult, mybir.AluOpType.add)
        nc.vector.tensor_tensor(t1[:], t1[:], t2[:], mybir.AluOpType.add)
        m2 = const.tile([128, RC, 1], f32)
        nc.vector.tensor_reduce(m2[:], t1[:], mybir.AxisListType.X,
                                mybir.AluOpType.max)
        d12 = const.tile([128, RC, 1], f32)
        nc.vector.tensor_tensor(d12[:], m1[:], m2[:], mybir.AluOpType.subtract)
        whi = const.tile([128, RC, 1], f32)
        nc.scalar.activation(whi[:], d12[:], mybir.ActivationFunctionType.Sigmoid)
        wlo = const.tile([128, RC, 1], f32)
        nc.vector.tensor_scalar(wlo[:], whi[:], -1.0, 1.0,
                                mybir.AluOpType.mult, mybir.AluOpType.add)
        mask1 = const.tile(shp, f32)
        nc.vector.tensor_tensor(mask1[:], logits[:], m1[:].to_broadcast(shp),
                                mybir.AluOpType.is_ge)
        selm = const.tile(shp, f32)
        nc.vector.tensor_tensor(selm[:], logits[:], m2[:].to_broadcast(shp),
                                mybir.AluOpType.is_ge)
        mask2 = const.tile(shp, f32)
        nc.vector.tensor_tensor(mask2[:], selm[:], mask1[:],
                                mybir.AluOpType.subtract)
        wd1 = const.tile(shp, f32)
        nc.vector.tensor_tensor(wd1[:], mask1[:], whi[:].to_broadcast(shp),
                                mybir.AluOpType.mult)
        wd2 = const.tile(shp, f32)
        nc.vector.tensor_tensor(wd2[:], mask2[:], wlo[:].to_broadcast(shp),
                                mybir.AluOpType.mult)
        wdense = const.tile(shp, f32)
        nc.vector.tensor_tensor(wdense[:], wd1[:], wd2[:], mybir.AluOpType.add)

        wt_sb = const.tile([E, RC, 128], f32r)   # routing weights, [expert, token]
        for c in range(RC):
            tp = psum.tile([E, 128], f32, name="tp")
            nc.tensor.transpose(tp[:], wdense[:, c], ident[:])
            nc.vector.tensor_copy(wt_sb[:, c], tp[:])

        psum_r.__exit__(None, None, None)

        # ---- phase M: experts (0 = shared, 1..8 = routed)
        psum = ctx.enter_context(tc.tile_pool(name="ps_m", bufs=1, space="PSUM"))
        for half in range(NHALF):
            xr = xpool.tile([128, KH, HALF], f32r, tag="xr")
            nc.sync.dma_start(xr[:], xtr_d[:, :, half * HALF:(half + 1) * HALF])
            y = ypool.tile([128, HT, HALF], f32, tag="y")
            for e in range(E + 1):
                wg = wpool.tile([128, KH, I], f32r, tag="wg")
                nc.sync.dma_start(wg[:], wg_d[e])
                wu = wpool.tile([128, KH, I], f32r, tag="wu")
                nc.sync.dma_start(wu[:], wu_d[e])
                wd = wpool.tile([128, KI, HP], f32r, tag="wd")
                nc.sync.dma_start(wd[:], wd_d[e])
                for c in range(HALF // CH):
                    gc = half * (HALF // CH) + c
                    tok = slice(c * CH, (c + 1) * CH)
                    if e > 0:
                        wb = psum.tile([128, CH], f32, name="wb")
                        nc.tensor.matmul(
                            wb[:], sel_s[:, e - 1],
                            wt_sb[:, gc * (CH // 128):(gc + 1) * (CH // 128)],
                            start=True, stop=True)
                    h = hpool.tile([128, IT, CH], f32r, tag="h")
                    for i in range(IT):
                        g_ps = psum.tile([128, CH], f32, name="g")
                        for k in range(KH):
                            nc.tensor.matmul(g_ps[:], wg[:, k, i * 128:(i + 1) * 128],
                                             xr[:, k, tok],
                                             start=(k == 0), stop=(k == KH - 1))
                        u_ps = psum.tile([128, CH], f32, name="u")
                        for k in range(KH):
                            nc.tensor.matmul(u_ps[:], wu[:, k, i * 128:(i + 1) * 128],
                                             xr[:, k, tok],
                                             start=(k == 0), stop=(k == KH - 1))
                        sg = spool.tile([128, CH], f32, tag="sg")
                        act = (mybir.ActivationFunctionType.Sigmoid
                               if _SILU_SUB_SIGMOID
                               else mybir.ActivationFunctionType.Silu)
                        nc.scalar.activation(sg[:], g_ps[:], act)
                        if e == 0:
                            nc.vector.tensor_tensor(h[:, i], sg[:], u_ps[:],
                                                    mybir.AluOpType.mult)
                        else:
                            hx = spool.tile([128, CH], f32, tag="hx")
                            nc.vector.tensor_tensor(hx[:], sg[:], u_ps[:],
                                                    mybir.AluOpType.mult)
                            nc.vector.tensor_tensor(h[:, i], hx[:], wb[:],
                                                    mybir.AluOpType.mult)
                    for j in range(HT):
                        yd = psum.tile([128, CH], f32, name=f"yd{j}")
                        for i in range(IT):
                            nc.tensor.matmul(yd[:], wd[:, i, j * 128:(j + 1) * 128],
                                             h[:, i],
                                             start=(i == 0), stop=(i == IT - 1))
                        if e == 0:
                            nc.vector.tensor_copy(y[:, j, tok], yd[:])
                        else:
                            nc.vector.tensor_tensor(y[:, j, tok], y[:, j, tok],
                                                    yd[:], mybir.AluOpType.add)
            nc.sync.dma_start(yt_d[:, :, half * HALF:(half + 1) * HALF], y[:])

    nc.compile()
    return nc


def _get_program():
    if "nc" not in _cached:
        _cached["nc"] = _build_program()
    return _cached["nc"]


def _shard_inputs(x, shared_gate_w, shared_up_w, shared_down_w,
                  routed_gate_w, routed_up_w, routed_down_w,
                  router_w, router_bias):
    """Build the 8 per-core input maps (host-side layout prep only)."""
    f = np.float32
    xf = np.ascontiguousarray(np.asarray(x, f).reshape(T, H))

    def stack_kxm(shared, routed, kdim, mdim):
        # [9, kdim(=H), mdim] zero-padded on K to KH*128, tiled to
        # [9, 128, ktiles, mdim]
        w = np.concatenate([np.asarray(shared, f)[None],
                            np.asarray(routed, f)], axis=0)
        kp = ((kdim + 127) // 128) * 128
        out = np.zeros((E + 1, kp, mdim), f)
        out[:, :kdim] = w
        kt = kp // 128
        return np.ascontiguousarray(
            out.reshape(E + 1, kt, 128, mdim).transpose(0, 2, 1, 3))

    wgall = stack_kxm(shared_gate_w, routed_gate_w, H, I)
    wuall = stack_kxm(shared_up_w, routed_up_w, H, I)

    dn = np.concatenate([np.asarray(shared_down_w, f)[None],
                         np.asarray(routed_down_w, f)], axis=0)  # [9, I, H]
    dpad = np.zeros((E + 1, I, HP), f)
    dpad[:, :, :H] = dn
    wdall = np.ascontiguousarray(
        dpad.reshape(E + 1, KI, 128, HP).transpose(0, 2, 1, 3))

    rw = np.zeros((HP, E), f)
    rw[:H] = np.asarray(router_w, f)
    rw[H] = np.asarray(router_bias, f)          # bias row (x row 576 == 1.0)
    wrf = np.ascontiguousarray(rw.reshape(KH, 128, E).transpose(1, 0, 2))

    selmat = np.zeros((E, E, 128), f)
    for e in range(E):
        selmat[e, e, :] = 1.0

    in_maps = []
    for c in range(NCORES):
        xs = xf[c * TL:(c + 1) * TL]            # [TL, H]
        xp = np.zeros((TL, HP), f)
        xp[:, :H] = xs
        xp[:, H] = 1.0                          # bias row for the router
        xt = np.ascontiguousarray(
            xp.T.reshape(KH, 128, TL).transpose(1, 0, 2))
        in_maps.append({
            "xtf": xt, "xtr": xt,
            "wgall": wgall, "wuall": wuall, "wdall": wdall,
            "wrf": wrf, "selmat": selmat,
        })
    return in_maps


def _assemble_output(core_outs):
    y = np.empty((T, H), np.float32)
    for c in range(NCORES):
        yt = core_outs[c]["yt"]                 # [128, HT, TL]
        yh = yt.transpose(1, 0, 2).reshape(HP, TL)[:H]
        y[c * TL:(c + 1) * TL] = yh.T
    return y.reshape(B, S, H)


def kernel(**inputs):
    from concourse.bass_utils import run_bass_kernel_spmd
    nc = _get_program()
    in_maps = _shard_inputs(**inputs)
    res = run_bass_kernel_spmd(nc, in_maps, list(range(NCORES)))
    return _assemble_output(res.results)
